# revision 15
# baseline (speedup 1.0000x reference)
"""Trainium2 Bass kernel for nn_DivEncLayer (128 tiny per-slice MLPs).

Math: out[b,q] = sum_u W2[q,u] * elu(sum_s x[b,q,s]*W1[q,s,u] + b1[q,u]) + b2[q]

Key identities used on-device (exact, not approximations):
    elu(z) = relu(z) + (min(exp(z),1) - 1)
so with A = relu(z+b1), Bt = min(exp(z+b1),1)-1:
    out[b,q] = sum_u W2[q,u]*(A_u + Bt_u) + b2[q]
Both branch tensors contract with the SAME weights W2, so layer 2 is two
accumulating PE matmuls into one PSUM slot per (q, sample-chunk).

Device mapping (per core, batch-sharded 8 ways):
  - x is host-pre-transposed to [(q,s)=1024 rows, B] bf16 so the contraction
    dim (s) lies on SBUF partitions.  Layer 1 runs as 16-way concurrent
    32x32 tile_position-packed matmuls (one tiny MLP slice per PE tile).
  - z lands in PSUM in groups of 2 banks = 8 q-slices x 512 samples.
  - ScalarE drains exp(z) (PSUM->SBUF bf16); relu(z) drains on VectorE
    or ScalarE; the min/-1 runs on VectorE (bf16 4x) or GPSIMD.
  - Layer 2: per z-bank K=128 matmuls whose lhsT is a [128, 32] W2 column
    block (4 nonzero columns); 16 matmuls accumulate each dense 32-row
    output slab, giving a fully dense [128 q, 512 b] PSUM bank per chunk.
    One VectorE copy evacuates it, DMA'd out as out^T.  Host undoes the
    row permutation and transpose when assembling the full output.
  - Engine balance per 16 groups/chunk: exp on ScalarE (all), relu on
    VectorE (14) / ScalarE (2), min-part on GPSIMD (12) / VectorE (4).
  - Walrus in this toolchain encodes only ONE sync-wait per instruction;
    _split_multi_waits hoists extras onto same-engine NoOps.
"""

import os
import sys

import numpy as np

for _p in ("/opt/trn_rl_repo", "/root/.axon_site/_ro/trn_rl_repo"):
    if os.path.isdir(_p) and _p not in sys.path:
        sys.path.append(_p)

from contextlib import ExitStack

from concourse import bass, mybir, tile
from concourse.bass_utils import run_bass_kernel_spmd

B, Q, S, U = 65536, 128, 8, 32
NCORES = 8
BC = B // NCORES  # 8192 samples per core
CHUNK = 512       # samples per pipeline chunk
BF16 = mybir.dt.bfloat16
F32 = mybir.dt.float32
NPBF16 = mybir.dt.np(BF16)


def _split_multi_waits(nc):
    """Walrus codegen in this toolchain only encodes ONE sync-wait per
    instruction.  Hoist extra waits onto preceding same-engine NoOps."""
    k = 0
    for b in nc.main_func.blocks:
        il = b.instructions
        out = []
        for ins in il:
            si = ins.sync_info
            if si is not None and si.on_wait and len(si.on_wait) > 1:
                waits = list(si.on_wait)
                for w in waits[:-1]:
                    k += 1
                    nop = mybir.InstNoOp(
                        name=f"wsplit_{k}_{ins.name}",
                        engine=ins.engine,
                        ins=[],
                        outs=[],
                        sync_info=mybir.SyncInfo(on_wait=[w], on_update=[]),
                    )
                    nc.register_instruction(nop, overwrite=True)
                    out.append(nop)
                ins.sync_info = mybir.SyncInfo(
                    on_wait=[waits[-1]], on_update=list(si.on_update or [])
                )
            out.append(ins)
        b.instructions = out


def build_nc(bc=BC, chunk=CHUNK, act_a_groups=(7, 15),
             gp_bt_groups=(0, 1, 2, 3, 4, 5, 8, 9, 10, 11, 12, 13),
             has_b1=False, has_b2=False):
    """Build the single-core Bass program (run SPMD on all 8 cores)."""
    assert bc % chunk == 0
    nchunk = bc // chunk
    nc = bass.Bass()
    AT = mybir.ActivationFunctionType
    OP = mybir.AluOpType

    xt = nc.declare_dram_parameter("xt", [Q * S, bc], BF16, isOutput=False)
    w1s = nc.declare_dram_parameter("w1s", [128, 1024], BF16, isOutput=False)
    w2s = nc.declare_dram_parameter("w2s", [128, 1024], BF16, isOutput=False)
    if has_b1:
        b1p = nc.declare_dram_parameter("b1s", [128, 32], F32, isOutput=False)
    if has_b2:
        b2p = nc.declare_dram_parameter("b2s", [128, 128], BF16, isOutput=False)
    outt = nc.declare_dram_parameter("outt", [Q, bc], F32, isOutput=True)

    with tile.TileContext(nc) as tc, ExitStack() as ctx:
        wpool = ctx.enter_context(tc.tile_pool(name="w", bufs=1))
        xpool = ctx.enter_context(tc.tile_pool(name="x", bufs=12))
        zpool = ctx.enter_context(tc.tile_pool(name="zp", bufs=3, space="PSUM"))
        l2pool = ctx.enter_context(tc.tile_pool(name="l2p", bufs=2, space="PSUM"))
        epool = ctx.enter_context(tc.tile_pool(name="e", bufs=4))
        apool = ctx.enter_context(tc.tile_pool(name="a", bufs=32))
        bpool = ctx.enter_context(tc.tile_pool(name="b", bufs=32))
        opool = ctx.enter_context(tc.tile_pool(name="o", bufs=3))

        w1sb = wpool.tile([128, 1024], BF16, name="w1sb")
        nc.sync.dma_start(w1sb[:], w1s[:])
        w2sb = wpool.tile([128, 1024], BF16, name="w2sb")
        nc.sync.dma_start(w2sb[:], w2s[:])
        if has_b1:
            b1sb = wpool.tile([128, 32], F32, name="b1sb")
            nc.sync.dma_start(b1sb[:], b1p[:])
        if has_b2:
            b2sb = wpool.tile([128, 128], BF16, name="b2sb")
            nc.sync.dma_start(b2sb[:], b2p[:])
            ones = wpool.tile([128, chunk], BF16, name="ones")
            nc.vector.memset(ones[:], 1.0)

        def emit_l1_group(c, g, xts, abl):
            """Layer 1 for group g (8 q-slices x chunk samples) + ELU drains."""
            p, kk = g // 2, g % 2
            z = zpool.tile([128, 2 * chunk], F32, tag="z", name=f"z_{c}_{g}")
            for half in range(2):
                i = 2 * kk + half
                for j in range(4):
                    # q = 16p + 4i + j ; out strip j of bank `half`
                    nc.tensor.matmul(
                        z[32 * j:32 * j + 32, half * chunk:(half + 1) * chunk],
                        w1sb[32 * i:32 * i + 32,
                             128 * p + 32 * j:128 * p + 32 * j + 32],
                        xts[p][32 * i:32 * i + 32, :],
                        start=True, stop=True,
                        tile_position=(32 * i, 32 * j),
                    )
            E = epool.tile([128, 2 * chunk], BF16, tag="E", name=f"E_{c}_{g}")
            A = apool.tile([128, 2 * chunk], BF16, tag="A", name=f"A_{c}_{g}")
            Bt = bpool.tile([128, 2 * chunk], BF16, tag="B", name=f"B_{c}_{g}")
            a_on_act = g in act_a_groups
            # The DVE A-pass reads the z bank BEFORE the ACT E-pass: the
            # same-bank serialization dep then lands on the ACT instruction
            # (2 wait slots) instead of the TensorScalar (1 wait slot).
            if has_b1:
                # per-bank passes: bias vector differs between the two banks
                for half in range(2):
                    i = 2 * kk + half
                    col = 4 * p + i
                    sl = (slice(0, 128), slice(half * chunk, (half + 1) * chunk))
                    bias = b1sb[:, col:col + 1]
                    if a_on_act:
                        nc.scalar.activation(A[sl], z[sl], AT.Relu, bias=bias)
                    else:
                        nc.vector.tensor_scalar(
                            A[sl], z[sl], bias, 0.0, OP.add, OP.max)
                    nc.scalar.activation(E[sl], z[sl], AT.Exp, bias=bias)
            else:
                if a_on_act:
                    nc.scalar.activation(A[:], z[:], AT.Relu)
                else:
                    nc.vector.tensor_scalar_max(A[:], z[:], 0.0)
                nc.scalar.activation(E[:], z[:], AT.Exp)
            # Bt = min(E,1) - 1   (bf16 SBUF, 4x mode on DVE)
            bt_eng = nc.gpsimd if g in gp_bt_groups else nc.vector
            bt_eng.tensor_scalar(Bt[:], E[:], 1.0, 1.0, OP.min, OP.subtract)
            abl.append((A, Bt))

        def emit_l2_chunk_mms(l2, abl, step):
            """Emit the 4 layer-2 matmuls of sub-step `step` (0..15).

            Layer 2 runs as K=128 matmuls: for each z-bank (group g, half h)
            and each elu branch, one matmul with lhsT = w2k column block
            (4 nonzero W2 columns) accumulates into the dense output slab
            l2[32m:32m+32, :] where m = p//2.  64 MMs per chunk total.
            """
            g = step
            p, kk = g // 2, g % 2
            m = p // 2
            for part in range(2):
                for h in range(2):
                    i = 2 * kk + h
                    src_t = abl[g][part]
                    cb = (4 * p + i) * 32
                    nc.tensor.matmul(
                        l2[32 * m:32 * m + 32, :],
                        w2sb[:, cb:cb + 32],
                        src_t[:, h * chunk:(h + 1) * chunk],
                        start=(g % 4 == 0 and part == 0 and h == 0),
                        stop=(g % 4 == 3 and part == 1 and h == 1
                              and not has_b2),
                        skip_group_check=True,
                        tile_position=(0, 32 * m),
                    )
            if has_b2 and g % 4 == 3:
                nc.tensor.matmul(
                    l2[32 * m:32 * m + 32, :],
                    b2sb[0:1, 32 * m:32 * m + 32],
                    ones[0:1, :],
                    start=False, stop=True,
                    skip_group_check=True,
                    tile_position=(0, 32 * m),
                )

        def emit_store(c, l2):
            osb = opool.tile([128, chunk], F32, tag="o", name=f"o_{c}")
            nc.vector.tensor_copy(osb[:], l2[:])
            nc.sync.dma_start(outt[:, c * chunk:(c + 1) * chunk], osb[:])

        prev_abl = None
        for c in range(nchunk):
            xts = []
            for p in range(8):
                xtile = xpool.tile([128, chunk], BF16, tag="x", name=f"x_{c}_{p}")
                nc.sync.dma_start(
                    xtile[:], xt[128 * p:128 * (p + 1),
                                 c * chunk:(c + 1) * chunk])
                xts.append(xtile)
            abl = []
            if prev_abl is not None:
                prev_l2 = l2pool.tile([128, chunk], F32, tag="l2",
                                      name=f"l2_{c - 1}")
            for g in range(16):
                emit_l1_group(c, g, xts, abl)
                if prev_abl is not None:
                    emit_l2_chunk_mms(prev_l2, prev_abl, g)
            if prev_abl is not None:
                emit_store(c - 1, prev_l2)
            prev_abl = abl
        # tail: layer 2 of the last chunk
        l2 = l2pool.tile([128, chunk], F32, tag="l2", name=f"l2_{nchunk - 1}")
        for g in range(16):
            emit_l2_chunk_mms(l2, prev_abl, g)
        emit_store(nchunk - 1, l2)

    _split_multi_waits(nc)
    return nc


def prep_weights(W1, b1, W2, b2):
    """Host-side weight layouts matching the device q <-> (p,i,j) mapping."""
    W1 = np.asarray(W1, np.float32).reshape(Q, S, U)
    W2 = np.asarray(W2, np.float32).reshape(Q, U)
    b1 = np.asarray(b1, np.float32).reshape(Q, U)
    b2 = np.asarray(b2, np.float32).reshape(Q)
    w1s = np.zeros((128, 1024), np.float32)
    w2s = np.zeros((128, 1024), np.float32)
    b1s = np.zeros((128, 32), np.float32)
    b2s = np.zeros((128, 128), np.float32)
    perm = np.zeros(128, np.int64)
    for p in range(8):
        for i in range(4):
            for j in range(4):
                q = 16 * p + 4 * i + j
                lam = 16 * (p % 2) + 4 * i + j
                m = p // 2
                w1s[32 * i + 8 * j:32 * i + 8 * j + 8,
                    128 * p + 32 * j:128 * p + 32 * j + 32] = W1[q]
                w2s[32 * j:32 * j + 32, (4 * p + i) * 32 + lam] = W2[q]
                b1s[32 * j:32 * j + 32, 4 * p + i] = b1[q]
                b2s[0, 32 * m + lam] = b2[q]
                perm[32 * m + lam] = q
    return (w1s.astype(NPBF16), w2s.astype(NPBF16), b1s,
            b2s.astype(NPBF16), perm)


_NC_CACHE = {}


def _get_nc(has_b1, has_b2):
    key = (has_b1, has_b2)
    if key not in _NC_CACHE:
        _NC_CACHE[key] = build_nc(has_b1=has_b1, has_b2=has_b2)
    return _NC_CACHE[key]


def run(x, W1, b1, W2, b2, trace=False):
    x = np.asarray(x, np.float32).reshape(B, Q * S)
    b1 = np.asarray(b1, np.float32)
    b2 = np.asarray(b2, np.float32)
    has_b1 = bool(np.any(b1))
    has_b2 = bool(np.any(b2))
    nc = _get_nc(has_b1, has_b2)
    w1s, w2s, b1s, b2s, perm = prep_weights(W1, b1, W2, b2)
    xt_full = np.ascontiguousarray(x.astype(NPBF16).T)  # [1024, B]
    in_maps = []
    for c in range(NCORES):
        m = {
            "xt": np.ascontiguousarray(xt_full[:, c * BC:(c + 1) * BC]),
            "w1s": w1s,
            "w2s": w2s,
        }
        if has_b1:
            m["b1s"] = b1s
        if has_b2:
            m["b2s"] = b2s
        in_maps.append(m)
    res = run_bass_kernel_spmd(nc, in_maps, list(range(NCORES)), trace=trace)
    out = np.empty((B, Q), np.float32)
    for c in range(NCORES):
        ot = np.asarray(res.results[c]["outt"], np.float32)  # [128, BC]
        out[c * BC:(c + 1) * BC, :][:, perm] = ot.T
    return out, res


def kernel(x, W1, b1, W2, b2):
    out, _ = run(x, W1, b1, W2, b2, trace=False)
    return out
